# revision 7
# baseline (speedup 1.0000x reference)
"""Block-sparse attention (block-local) Bass kernel for 8 Trainium2 NeuronCores.

Problem: x[4, 4096, 1024] -> 4 linear projections (Q/K/V/O) + block-local
attention (block size 128, 16 heads, d_k 64), all f32.

Sharding: pure data parallel over tokens. Attention is block-local with
block size 128, so the flattened token axis [16384] splits across 8 cores
into 2048-token shards (16 blocks each) with zero cross-core communication.

Per-core kernel layout strategy:
 - All DRAM tensors are host-pre-arranged so every DMA descriptor is a long
   contiguous run per partition (2-16 KB). Scattered small descriptors keep
   the DMA engines active for most of the kernel, and concurrent DMA traffic
   steals SBUF bandwidth from the PE's rhs/weight streams (measured: 259 ns
   vs 216 ns per N=512 matmul).
 - x is passed host-transposed and pre-tiled as xt[128, 4, 8, 512] so
   activations live in SBUF with d_model on partitions; Q/K projections then
   need no on-chip transposes.
 - Wq/Wk are passed as [128, 8m, 8c, 128] (m-chunk-major: per-m DMAs are
   contiguous); Wv/Wo as [128, 8c, 1024n] (single contiguous DMA, n-span
   rhs slices stay contiguous).
 - Q^T/K^T produced in [d_model, token] layout (what scores matmuls need),
   V in natural [token, d_model] layout (what the A@V matmul needs).
 - Per 128-token block: scores -> exp -> row-sum -> normalize (all in
   natural [q, k] layout, reductions along free dim), then PE-transpose of
   A to feed A@V, whose [d, q] output is exactly the lhsT the final Wo
   projection needs. Output bias bo is added by the DVE during the
   PSUM->SBUF drain (broadcast tile built on-chip by GpSimd) - no PE cost.
 - Output is stored fp16 (half the store traffic; tolerance allows it) and
   upcast to f32 on the host.
"""
import sys

if '/opt/trn_rl_repo' not in sys.path:
    sys.path.insert(0, '/opt/trn_rl_repo')

import numpy as np

import concourse.bass as bass
import concourse.mybir as mybir
import concourse.tile as tile
from concourse.vector_clock import ScopedClock
from concourse.masks import make_identity
from concourse.bass_utils import run_bass_kernel_spmd

F32 = mybir.dt.float32
BF16 = mybir.dt.float16  # attention-path dtype (fp16: same PE rate, more mantissa)

D = 1024          # d_model
NH = 16           # heads
DK = 64           # head dim
BS = 128          # attention block size
N_CORES = 8
TOK = 2048        # tokens per core
ST = 512          # supertile tokens
NST = TOK // ST   # supertiles per core
SCALE = 1.0 / 8.0  # 1/sqrt(DK)

_MAX_DRAIN_WAITS = 1


class _SplitDrainTileContext(tile.TileContext):
    """The walrus in this container rejects >1 sync-wait on a NO_STRUCT
    instruction; Tile's exit drain waits on the whole global clock. Spread
    the waits across a chain of drains."""

    def _drain_and_barrier(self, tick_clock, wait_clock):
        nc = self.nc
        probe = nc.sync.drain()
        wait_clock.add_sem_waits(probe.ins, ScopedClock({None: tick_clock.global_clock}))
        si = probe.ins.sync_info
        waits = list(si.on_wait) if (si and si.on_wait) else []
        if len(waits) > _MAX_DRAIN_WAITS:
            probe.ins.sync_info = mybir.SyncInfo(
                on_wait=waits[:_MAX_DRAIN_WAITS],
                on_update=list(si.on_update) if si.on_update else [],
            )
            for i in range(_MAX_DRAIN_WAITS, len(waits), _MAX_DRAIN_WAITS):
                d = nc.sync.drain()
                d.ins.sync_info = mybir.SyncInfo(
                    on_wait=waits[i:i + _MAX_DRAIN_WAITS], on_update=[]
                )
        nc.all_engine_barrier()
        assert self.sems is not None
        popped = nc._tile_sem_poison_stack.pop()
        assert popped is self._sem_poison
        nc.clear_and_free_semaphores(list(self.sems.allocated().values()))
        nc.all_engine_barrier()


def _split_excess_waits(nc, limit=1):
    """The nix walrus rejects instructions carrying more than `limit` sync
    waits. Hoist excess waits onto EventSemaphore instructions inserted just
    before, on the same (in-order) engine — semantics preserved."""
    n_split = 0
    for f in nc.m.functions:
        for bb in f.blocks:
            new = []
            changed = False
            for inst in bb.instructions:
                si = inst.sync_info
                waits = list(si.on_wait) if (si and si.on_wait) else []
                if len(waits) > limit:
                    excess = waits[:-limit]
                    for i in range(0, len(excess), limit):
                        ev = mybir.InstEventSemaphore(
                            name=f'I-splitw-{nc.next_id()}')
                        ev.engine = inst.engine
                        ev.sync_info = mybir.SyncInfo(
                            on_wait=excess[i:i + limit], on_update=[])
                        new.append(ev)
                        n_split += 1
                    inst.sync_info = mybir.SyncInfo(
                        on_wait=waits[-limit:],
                        on_update=list(si.on_update) if si.on_update else [])
                    changed = True
                new.append(inst)
            if changed:
                bb.instructions = new
    return n_split


def build_bass(split_waits=True):
    nc = bass.Bass('TRN2', target_bir_lowering=False, num_devices=N_CORES)

    xt_d = nc.dram_tensor('xt', [128, NST, 8, ST], BF16, kind='ExternalInput')
    wq_d = nc.dram_tensor('wq', [128, 8, 8, 128], BF16, kind='ExternalInput')
    wk_d = nc.dram_tensor('wk', [128, 8, 8, 128], BF16, kind='ExternalInput')
    wv_d = nc.dram_tensor('wv', [128, 8, D], BF16, kind='ExternalInput')
    wo_d = nc.dram_tensor('wo', [128, 8, D], BF16, kind='ExternalInput')
    bq_d = nc.dram_tensor('bq', [128, 8], F32, kind='ExternalInput')
    bk_d = nc.dram_tensor('bk', [128, 8], F32, kind='ExternalInput')
    bv_d = nc.dram_tensor('bv', [1, D], F32, kind='ExternalInput')
    bo_d = nc.dram_tensor('bo', [1, D], F32, kind='ExternalInput')
    out_d = nc.dram_tensor('out', [TOK, D], BF16, kind='ExternalOutput')

    with _SplitDrainTileContext(nc) as tc:
        _build_body(nc, tc, xt_d, wq_d, wk_d, wv_d, wo_d,
                    bq_d, bk_d, bv_d, bo_d, out_d)
    if split_waits:
        # CoreSim chokes on the inserted EventSemaphores; only split for HW.
        _split_excess_waits(nc, limit=1)
    return nc


def _build_body(nc, tc, xt_d, wq_d, wk_d, wv_d, wo_d, bq_d, bk_d, bv_d, bo_d, out_d):
    from contextlib import ExitStack
    with ExitStack() as ctx:
        _build_pools_and_body(nc, tc, ctx, xt_d, wq_d, wk_d, wv_d, wo_d,
                              bq_d, bk_d, bv_d, bo_d, out_d)


def _build_pools_and_body(nc, tc, ctx, xt_d, wq_d, wk_d, wv_d, wo_d,
                          bq_d, bk_d, bv_d, bo_d, out_d):
    AF = mybir.ActivationFunctionType
    OP = mybir.AluOpType
    AX = mybir.AxisListType

    wpool = ctx.enter_context(tc.tile_pool(name='w', bufs=1))
    cpool = ctx.enter_context(tc.tile_pool(name='c', bufs=1))
    xpool = ctx.enter_context(tc.tile_pool(name='x', bufs=2))
    qkv = ctx.enter_context(tc.tile_pool(name='qkv', bufs=2))
    apool = ctx.enter_context(tc.tile_pool(name='a', bufs=2))
    opool = ctx.enter_context(tc.tile_pool(name='o', bufs=2))
    otpool = ctx.enter_context(tc.tile_pool(name='ot', bufs=2))

    pp = ctx.enter_context(tc.tile_pool(name='pp', bufs=2, space='PSUM'))
    pat = ctx.enter_context(tc.tile_pool(name='pat', bufs=2, space='PSUM'))
    psc = ctx.enter_context(tc.tile_pool(name='psc', bufs=2, space='PSUM'))
    pav = ctx.enter_context(tc.tile_pool(name='pav', bufs=1, space='PSUM'))

    # ---- constants / weights ----
    # First supertile's activations go first so the PE can start early;
    # weights stream in right behind.
    # DMA issue order = criticality order: the first Q-proj matmul needs
    # xt0 + wq[m0]; everything else (wk/wv/wo, later xt tiles, bias
    # broadcasts) is needed 15-60us later and must not crowd the queues.
    xt_tiles = [None] * NST
    xt_tiles[0] = xpool.tile([128, 8, ST], BF16, name='xt')
    nc.sync.dma_start(out=xt_tiles[0], in_=xt_d.ap()[:, 0])

    w_sb = {}
    w_sb['q'] = wpool.tile([128, 8, 8, 128], BF16, name='wq')
    w_sb['k'] = wpool.tile([128, 8, 8, 128], BF16, name='wk')
    w_sb['v'] = wpool.tile([128, 8, D], BF16, name='wv')
    w_sb['o'] = wpool.tile([128, 8, D], BF16, name='wo')
    for m in range(8):
        nc.sync.dma_start(out=w_sb['q'][:, m], in_=wq_d.ap()[:, m])

    bq_sb = cpool.tile([128, 8], F32, name='bq')
    nc.sync.dma_start(out=bq_sb, in_=bq_d.ap())
    bk_sb = cpool.tile([128, 8], F32, name='bk')
    nc.sync.dma_start(out=bk_sb, in_=bk_d.ap())

    nc.sync.dma_start(out=w_sb['k'], in_=wk_d.ap())
    nc.sync.dma_start(out=w_sb['v'], in_=wv_d.ap())
    nc.sync.dma_start(out=w_sb['o'], in_=wo_d.ap())

    # bias rows, broadcast across partitions by a stride-0 DMA (contiguous
    # 4KB run per partition; walrus here lacks gpsimd partition_broadcast)
    bv_ap = bv_d.ap()
    bv_bc = cpool.tile([128, D], F32, name='bvbc')
    nc.sync.dma_start(
        out=bv_bc,
        in_=bass.AP(tensor=bv_ap.tensor, offset=bv_ap.offset,
                    ap=[[0, 128], [1, D]]))
    bo_ap = bo_d.ap()
    bo_bc = cpool.tile([128, D], F32, name='bobc')
    nc.sync.dma_start(
        out=bo_bc,
        in_=bass.AP(tensor=bo_ap.tensor, offset=bo_ap.offset,
                    ap=[[0, 128], [1, D]]))

    ident = cpool.tile([128, 128], BF16, name='ident')
    make_identity(nc, ident)

    # PE warm-up: HAM un-throttles only after ~3.4us of sustained activity.
    # Run dummy matmuls on a memset tile while the first DMAs land so the
    # real matmul stream starts at 2.4 GHz.
    warm_sb = cpool.tile([128, 512], BF16, name='warm')
    nc.vector.memset(warm_sb, 0.5)
    ps_warm = pp.tile([128, ST], F32, name='ps')
    for _ in range(36):
        nc.tensor.matmul(ps_warm, lhsT=warm_sb[:, 0:128], rhs=warm_sb,
                         start=True, stop=True)

    for s in range(NST):
        if xt_tiles[s] is None:
            xt_tiles[s] = xpool.tile([128, 8, ST], BF16, name='xt')
            nc.sync.dma_start(out=xt_tiles[s], in_=xt_d.ap()[:, s])
        xt_sb = xt_tiles[s]
        if s + 1 < NST:
            # prefetch next supertile's activations; lands during this
            # supertile's attention phase (xpool is double-buffered)
            xt_tiles[s + 1] = xpool.tile([128, 8, ST], BF16, name='xt')
            nc.sync.dma_start(out=xt_tiles[s + 1], in_=xt_d.ap()[:, s + 1])

        # ---- projections ----
        qt_sb = qkv.tile([128, 8, ST], BF16, name='qt')
        kt_sb = qkv.tile([128, 8, ST], BF16, name='kt')
        v_sb = qkv.tile([128, 4, D], BF16, name='v')

        for m in range(8):
            ps = pp.tile([128, ST], F32, name='ps')
            for c in range(8):
                nc.tensor.matmul(ps, lhsT=w_sb['q'][:, m, c, :],
                                 rhs=xt_sb[:, c, :], start=(c == 0), stop=(c == 7))
            nc.vector.tensor_scalar(out=qt_sb[:, m, :], in0=ps,
                                    scalar1=bq_sb[:, m:m + 1], scalar2=SCALE,
                                    op0=OP.add, op1=OP.mult)
        for m in range(8):
            ps = pp.tile([128, ST], F32, name='ps')
            for c in range(8):
                nc.tensor.matmul(ps, lhsT=w_sb['k'][:, m, c, :],
                                 rhs=xt_sb[:, c, :], start=(c == 0), stop=(c == 7))
            nc.vector.tensor_scalar(out=kt_sb[:, m, :], in0=ps,
                                    scalar1=bk_sb[:, m:m + 1], scalar2=None,
                                    op0=OP.add)
        for tch in range(4):
            for nh2 in range(2):
                ps = pp.tile([128, ST], F32, name='ps')
                for c in range(8):
                    nc.tensor.matmul(
                        ps, lhsT=xt_sb[:, c, tch * 128:(tch + 1) * 128],
                        rhs=w_sb['v'][:, c, nh2 * 512:(nh2 + 1) * 512],
                        start=(c == 0), stop=(c == 7))
                nc.vector.tensor_tensor(
                    out=v_sb[:, tch, nh2 * 512:(nh2 + 1) * 512], in0=ps,
                    in1=bv_bc[:, nh2 * 512:(nh2 + 1) * 512], op=OP.add)

        # ---- attention + output projection, per 128-token block ----
        for b4 in range(4):
            t0 = b4 * 128
            ps_av0 = pav.tile([128, 4, 128], F32, name='ps_av0')
            ps_av1 = pav.tile([128, 4, 128], F32, name='ps_av1')
            for g in range(4):
                # Heads grouped by parity: every scores matmul in this group
                # reads Q^T/K^T at the SAME partition offset. Mixing partition
                # offsets across matmuls that write one PSUM bank wedges the
                # device (HW/codegen bug), so each bank sees one offset only.
                parity = g % 2
                base = (g // 2) * 8
                heads = [base + parity + 2 * i for i in range(4)]
                off = parity * 64
                ps_sc = psc.tile([128, 4, 128], F32, name='ps_sc')
                for i, hh in enumerate(heads):
                    m = hh // 2
                    nc.tensor.matmul(
                        ps_sc[:, i, :],
                        lhsT=qt_sb[off:off + 64, m, t0:t0 + 128],
                        rhs=kt_sb[off:off + 64, m, t0:t0 + 128],
                        start=True, stop=True)
                e_sb = apool.tile([128, 4, 128], BF16, name='e')
                nc.scalar.activation(e_sb, ps_sc, AF.Exp)
                stat = apool.tile([128, 8], F32, name='stat')
                nc.vector.reduce_sum(out=stat[:, 0:4], in_=e_sb, axis=AX.X)
                nc.vector.reciprocal(stat[:, 4:8], stat[:, 0:4])
                nc.vector.tensor_tensor(out=e_sb, in0=e_sb,
                                        in1=stat[:, 4:8].to_broadcast((128, 4, 128)),
                                        op=OP.mult)
                ps_at = pat.tile([128, 4, 128], BF16, name='ps_at')
                for i in range(4):
                    nc.tensor.transpose(ps_at[:, i, :], e_sb[:, i, :], ident)
                at_sb = apool.tile([128, 4, 128], BF16, name='at')
                nc.scalar.copy(at_sb, ps_at)
                for i, hh in enumerate(heads):
                    g2 = hh // 2
                    ps_av = ps_av0 if g2 < 4 else ps_av1
                    nc.tensor.matmul(
                        ps_av[off:off + 64, g2 % 4, :],
                        lhsT=v_sb[:, b4, hh * 64:(hh + 1) * 64],
                        rhs=at_sb[:, i, :],
                        start=True, stop=True)
                if g == 1:
                    # heads 0-7 done: start the O-projection's first half-
                    # contraction now so it overlaps attention groups 2-3
                    ot_sb = otpool.tile([128, 8, 128], BF16, name='ot')
                    nc.scalar.copy(ot_sb[:, 0:4, :], ps_av0)
                    ps_o0 = pp.tile([128, ST], F32, name='ps')
                    ps_o1 = pp.tile([128, ST], F32, name='ps')
                    for nh2, ps_o in ((0, ps_o0), (1, ps_o1)):
                        for c in range(4):
                            nc.tensor.matmul(
                                ps_o, lhsT=ot_sb[:, c, :],
                                rhs=w_sb['o'][:, c, nh2 * 512:(nh2 + 1) * 512],
                                start=(c == 0), stop=False)
                elif g == 3:
                    nc.scalar.copy(ot_sb[:, 4:8, :], ps_av1)

            for nh2, ps_o in ((0, ps_o0), (1, ps_o1)):
                for c in range(4, 8):
                    nc.tensor.matmul(
                        ps_o, lhsT=ot_sb[:, c, :],
                        rhs=w_sb['o'][:, c, nh2 * 512:(nh2 + 1) * 512],
                        start=False, stop=(c == 7))
                out_sb = opool.tile([128, 512], BF16, name='outsb')
                nc.vector.tensor_tensor(
                    out=out_sb, in0=ps_o,
                    in1=bo_bc[:, nh2 * 512:(nh2 + 1) * 512], op=OP.add)
                nc.sync.dma_start(
                    out=out_d.ap()[s * ST + t0: s * ST + t0 + 128,
                                   nh2 * 512:(nh2 + 1) * 512],
                    in_=out_sb)


_NC_CACHE = []


def _get_nc():
    if not _NC_CACHE:
        _NC_CACHE.append(build_bass())
    return _NC_CACHE[0]


def shard_inputs(x, Wq, bq, Wk, bk, Wv, bv, Wo, bo):
    x = np.asarray(x, dtype=np.float32)
    B, S, _ = x.shape
    xf = np.ascontiguousarray(x.reshape(B * S, D))
    assert B * S == N_CORES * TOK

    def qk_layout(W):
        # [in=1024, out=1024] -> [p=128, m=8, c=8, 128]; element [p,m,c,j] =
        # W[c*128+p, m*128+j]: per-(p,m) rows are contiguous 2KB DMA runs.
        return np.ascontiguousarray(
            np.asarray(W, dtype=np.float16).reshape(8, 128, 8, 128)
            .transpose(1, 2, 0, 3))

    def vo_layout(W):
        # [in, out] -> [p=128, c=8, n=1024]; per-partition rows contiguous.
        return np.ascontiguousarray(
            np.asarray(W, dtype=np.float16).reshape(8, 128, D)
            .transpose(1, 0, 2))

    shared = {
        'wq': qk_layout(Wq),
        'wk': qk_layout(Wk),
        'wv': vo_layout(Wv),
        'wo': vo_layout(Wo),
        'bq': np.ascontiguousarray(np.asarray(bq, dtype=np.float32).reshape(8, 128).T),
        'bk': np.ascontiguousarray(np.asarray(bk, dtype=np.float32).reshape(8, 128).T),
        'bv': np.ascontiguousarray(np.asarray(bv, dtype=np.float32).reshape(1, D)),
        'bo': np.ascontiguousarray(np.asarray(bo, dtype=np.float32).reshape(1, D)),
    }
    in_maps = []
    for c in range(N_CORES):
        # [tok=2048, d=1024] -> [p=128, s=4, c=8, t=512]; element [p,s,ch,t]
        # = x[s*512+t, ch*128+p]: per-(p,s) rows contiguous 8KB DMA runs.
        xs = xf[c * TOK:(c + 1) * TOK, :].astype(np.float16)
        xt = np.ascontiguousarray(
            xs.T.reshape(8, 128, NST, ST).transpose(1, 2, 0, 3))
        in_maps.append({'xt': xt, **shared})
    return (B, S), in_maps


def run(inputs, **spmd_kwargs):
    (B, S), in_maps = shard_inputs(**inputs)
    nc = _get_nc()
    res = run_bass_kernel_spmd(nc, in_maps, list(range(N_CORES)), **spmd_kwargs)
    out = np.concatenate([res.results[c]['out'] for c in range(N_CORES)], axis=0)
    return out.astype(np.float32).reshape(B, S, D), res


def kernel(x, Wq, bq, Wk, bk, Wv, bv, Wo, bo):
    out, _ = run(dict(x=x, Wq=Wq, bq=bq, Wk=Wk, bk=bk,
                      Wv=Wv, bv=bv, Wo=Wo, bo=bo))
    return out


# revision 8
# speedup vs baseline: 1.0433x; 1.0433x over previous
"""Block-sparse attention (block-local) Bass kernel for 8 Trainium2 NeuronCores.

Problem: x[4, 4096, 1024] -> 4 linear projections (Q/K/V/O) + block-local
attention (block size 128, 16 heads, d_k 64), all f32.

Sharding: pure data parallel over tokens. Attention is block-local with
block size 128, so the flattened token axis [16384] splits across 8 cores
into 2048-token shards (16 blocks each) with zero cross-core communication.

Per-core kernel layout strategy:
 - All DRAM tensors are host-pre-arranged so every DMA descriptor is a long
   contiguous run per partition (2-16 KB). Scattered small descriptors keep
   the DMA engines active for most of the kernel, and concurrent DMA traffic
   steals SBUF bandwidth from the PE's rhs/weight streams (measured: 259 ns
   vs 216 ns per N=512 matmul).
 - x is passed host-transposed and pre-tiled as xt[128, 4, 8, 512] so
   activations live in SBUF with d_model on partitions; Q/K projections then
   need no on-chip transposes.
 - Wq/Wk are passed as [128, 8m, 8c, 128] (m-chunk-major: per-m DMAs are
   contiguous); Wv/Wo as [128, 8c, 1024n] (single contiguous DMA, n-span
   rhs slices stay contiguous).
 - Q^T/K^T produced in [d_model, token] layout (what scores matmuls need),
   V in natural [token, d_model] layout (what the A@V matmul needs).
 - Per 128-token block: scores -> exp -> row-sum -> normalize (all in
   natural [q, k] layout, reductions along free dim), then PE-transpose of
   A to feed A@V, whose [d, q] output is exactly the lhsT the final Wo
   projection needs. Output bias bo is added by the DVE during the
   PSUM->SBUF drain (broadcast tile built on-chip by GpSimd) - no PE cost.
 - Output is stored fp16 (half the store traffic; tolerance allows it) and
   upcast to f32 on the host.
"""
import sys

if '/opt/trn_rl_repo' not in sys.path:
    sys.path.insert(0, '/opt/trn_rl_repo')

import numpy as np

import concourse.bass as bass
import concourse.mybir as mybir
import concourse.tile as tile
from concourse.vector_clock import ScopedClock
from concourse.masks import make_identity
from concourse.bass_utils import run_bass_kernel_spmd

F32 = mybir.dt.float32
BF16 = mybir.dt.float16  # attention-path dtype (fp16: same PE rate, more mantissa)

D = 1024          # d_model
NH = 16           # heads
DK = 64           # head dim
BS = 128          # attention block size
N_CORES = 8
TOK = 2048        # tokens per core
ST = 512          # supertile tokens
NST = TOK // ST   # supertiles per core
SCALE = 1.0 / 8.0  # 1/sqrt(DK)

_MAX_DRAIN_WAITS = 1


class _SplitDrainTileContext(tile.TileContext):
    """The walrus in this container rejects >1 sync-wait on a NO_STRUCT
    instruction; Tile's exit drain waits on the whole global clock. Spread
    the waits across a chain of drains."""

    def _drain_and_barrier(self, tick_clock, wait_clock):
        nc = self.nc
        probe = nc.sync.drain()
        wait_clock.add_sem_waits(probe.ins, ScopedClock({None: tick_clock.global_clock}))
        si = probe.ins.sync_info
        waits = list(si.on_wait) if (si and si.on_wait) else []
        if len(waits) > _MAX_DRAIN_WAITS:
            probe.ins.sync_info = mybir.SyncInfo(
                on_wait=waits[:_MAX_DRAIN_WAITS],
                on_update=list(si.on_update) if si.on_update else [],
            )
            for i in range(_MAX_DRAIN_WAITS, len(waits), _MAX_DRAIN_WAITS):
                d = nc.sync.drain()
                d.ins.sync_info = mybir.SyncInfo(
                    on_wait=waits[i:i + _MAX_DRAIN_WAITS], on_update=[]
                )
        nc.all_engine_barrier()
        assert self.sems is not None
        popped = nc._tile_sem_poison_stack.pop()
        assert popped is self._sem_poison
        nc.clear_and_free_semaphores(list(self.sems.allocated().values()))
        nc.all_engine_barrier()


def _split_excess_waits(nc, limit=1):
    """The nix walrus rejects instructions carrying more than `limit` sync
    waits. Hoist excess waits onto EventSemaphore instructions inserted just
    before, on the same (in-order) engine — semantics preserved."""
    n_split = 0
    for f in nc.m.functions:
        for bb in f.blocks:
            new = []
            changed = False
            for inst in bb.instructions:
                si = inst.sync_info
                waits = list(si.on_wait) if (si and si.on_wait) else []
                if len(waits) > limit:
                    excess = waits[:-limit]
                    for i in range(0, len(excess), limit):
                        ev = mybir.InstEventSemaphore(
                            name=f'I-splitw-{nc.next_id()}')
                        ev.engine = inst.engine
                        ev.sync_info = mybir.SyncInfo(
                            on_wait=excess[i:i + limit], on_update=[])
                        new.append(ev)
                        n_split += 1
                    inst.sync_info = mybir.SyncInfo(
                        on_wait=waits[-limit:],
                        on_update=list(si.on_update) if si.on_update else [])
                    changed = True
                new.append(inst)
            if changed:
                bb.instructions = new
    return n_split


def build_bass(split_waits=True):
    nc = bass.Bass('TRN2', target_bir_lowering=False, num_devices=N_CORES)

    xt_d = nc.dram_tensor('xt', [128, NST, 8, ST], BF16, kind='ExternalInput')
    wq_d = nc.dram_tensor('wq', [128, 8, 8, 128], BF16, kind='ExternalInput')
    wk_d = nc.dram_tensor('wk', [128, 8, 8, 128], BF16, kind='ExternalInput')
    wv_d = nc.dram_tensor('wv', [128, 8, D], BF16, kind='ExternalInput')
    wo_d = nc.dram_tensor('wo', [128, 8, D], BF16, kind='ExternalInput')
    bq_d = nc.dram_tensor('bq', [128, 8], F32, kind='ExternalInput')
    bk_d = nc.dram_tensor('bk', [128, 8], F32, kind='ExternalInput')
    bv_d = nc.dram_tensor('bv', [1, D], F32, kind='ExternalInput')
    bo_d = nc.dram_tensor('bo', [1, D], F32, kind='ExternalInput')
    out_d = nc.dram_tensor('out', [TOK, D], BF16, kind='ExternalOutput')

    with _SplitDrainTileContext(nc) as tc:
        _build_body(nc, tc, xt_d, wq_d, wk_d, wv_d, wo_d,
                    bq_d, bk_d, bv_d, bo_d, out_d)
    if split_waits:
        # CoreSim chokes on the inserted EventSemaphores; only split for HW.
        _split_excess_waits(nc, limit=1)
    return nc


def _build_body(nc, tc, xt_d, wq_d, wk_d, wv_d, wo_d, bq_d, bk_d, bv_d, bo_d, out_d):
    from contextlib import ExitStack
    with ExitStack() as ctx:
        _build_pools_and_body(nc, tc, ctx, xt_d, wq_d, wk_d, wv_d, wo_d,
                              bq_d, bk_d, bv_d, bo_d, out_d)


def _build_pools_and_body(nc, tc, ctx, xt_d, wq_d, wk_d, wv_d, wo_d,
                          bq_d, bk_d, bv_d, bo_d, out_d):
    AF = mybir.ActivationFunctionType
    OP = mybir.AluOpType
    AX = mybir.AxisListType

    wpool = ctx.enter_context(tc.tile_pool(name='w', bufs=1))
    cpool = ctx.enter_context(tc.tile_pool(name='c', bufs=1))
    xpool = ctx.enter_context(tc.tile_pool(name='x', bufs=2))
    qkv = ctx.enter_context(tc.tile_pool(name='qkv', bufs=2))
    apool = ctx.enter_context(tc.tile_pool(name='a', bufs=2))
    opool = ctx.enter_context(tc.tile_pool(name='o', bufs=2))
    otpool = ctx.enter_context(tc.tile_pool(name='ot', bufs=2))

    pp = ctx.enter_context(tc.tile_pool(name='pp', bufs=2, space='PSUM'))
    pat = ctx.enter_context(tc.tile_pool(name='pat', bufs=2, space='PSUM'))
    psc = ctx.enter_context(tc.tile_pool(name='psc', bufs=2, space='PSUM'))
    pav = ctx.enter_context(tc.tile_pool(name='pav', bufs=1, space='PSUM'))

    # ---- constants / weights ----
    # First supertile's activations go first so the PE can start early;
    # weights stream in right behind.
    # DMA issue order = criticality order: the first Q-proj matmul needs
    # xt0 + wq[m0]; everything else (wk/wv/wo, later xt tiles, bias
    # broadcasts) is needed 15-60us later and must not crowd the queues.
    xt_tiles = [None] * NST
    xt_tiles[0] = xpool.tile([128, 8, ST], BF16, name='xt')
    nc.sync.dma_start(out=xt_tiles[0], in_=xt_d.ap()[:, 0])

    w_sb = {}
    w_sb['q'] = wpool.tile([128, 8, 8, 128], BF16, name='wq')
    w_sb['k'] = wpool.tile([128, 8, 8, 128], BF16, name='wk')
    w_sb['v'] = wpool.tile([128, 8, D], BF16, name='wv')
    w_sb['o'] = wpool.tile([128, 8, D], BF16, name='wo')
    for m in range(8):
        nc.sync.dma_start(out=w_sb['q'][:, m], in_=wq_d.ap()[:, m])

    bq_sb = cpool.tile([128, 8], F32, name='bq')
    nc.sync.dma_start(out=bq_sb, in_=bq_d.ap())
    bk_sb = cpool.tile([128, 8], F32, name='bk')
    nc.sync.dma_start(out=bk_sb, in_=bk_d.ap())

    nc.sync.dma_start(out=w_sb['k'], in_=wk_d.ap())
    nc.sync.dma_start(out=w_sb['v'], in_=wv_d.ap())
    nc.sync.dma_start(out=w_sb['o'], in_=wo_d.ap())

    # bias rows, broadcast across partitions by a stride-0 DMA (contiguous
    # 4KB run per partition; walrus here lacks gpsimd partition_broadcast)
    bv_ap = bv_d.ap()
    bv_bc = cpool.tile([128, D], F32, name='bvbc')
    nc.sync.dma_start(
        out=bv_bc,
        in_=bass.AP(tensor=bv_ap.tensor, offset=bv_ap.offset,
                    ap=[[0, 128], [1, D]]))
    bo_ap = bo_d.ap()
    bo_bc = cpool.tile([128, D], F32, name='bobc')
    nc.sync.dma_start(
        out=bo_bc,
        in_=bass.AP(tensor=bo_ap.tensor, offset=bo_ap.offset,
                    ap=[[0, 128], [1, D]]))

    ident = cpool.tile([128, 128], BF16, name='ident')
    make_identity(nc, ident)

    # PE warm-up: HAM un-throttles only after ~3.4us of sustained activity.
    # Run dummy matmuls on a memset tile while the first DMAs land so the
    # real matmul stream starts at 2.4 GHz.
    warm_sb = cpool.tile([128, 512], BF16, name='warm')
    nc.vector.memset(warm_sb, 0.5)
    ps_warm = pp.tile([128, ST], F32, name='ps')
    for _ in range(22):
        nc.tensor.matmul(ps_warm, lhsT=warm_sb[:, 0:128], rhs=warm_sb,
                         start=True, stop=True)

    for s in range(NST):
        if xt_tiles[s] is None:
            xt_tiles[s] = xpool.tile([128, 8, ST], BF16, name='xt')
            nc.sync.dma_start(out=xt_tiles[s], in_=xt_d.ap()[:, s])
        xt_sb = xt_tiles[s]
        if s + 1 < NST:
            # prefetch next supertile's activations; lands during this
            # supertile's attention phase (xpool is double-buffered)
            xt_tiles[s + 1] = xpool.tile([128, 8, ST], BF16, name='xt')
            nc.sync.dma_start(out=xt_tiles[s + 1], in_=xt_d.ap()[:, s + 1])

        # ---- projections ----
        qt_sb = qkv.tile([128, 8, ST], BF16, name='qt')
        kt_sb = qkv.tile([128, 8, ST], BF16, name='kt')
        v_sb = qkv.tile([128, 4, D], BF16, name='v')

        for m in range(8):
            ps = pp.tile([128, ST], F32, name='ps')
            for c in range(8):
                nc.tensor.matmul(ps, lhsT=w_sb['q'][:, m, c, :],
                                 rhs=xt_sb[:, c, :], start=(c == 0), stop=(c == 7))
            nc.vector.tensor_scalar(out=qt_sb[:, m, :], in0=ps,
                                    scalar1=bq_sb[:, m:m + 1], scalar2=SCALE,
                                    op0=OP.add, op1=OP.mult)
        for m in range(8):
            ps = pp.tile([128, ST], F32, name='ps')
            for c in range(8):
                nc.tensor.matmul(ps, lhsT=w_sb['k'][:, m, c, :],
                                 rhs=xt_sb[:, c, :], start=(c == 0), stop=(c == 7))
            nc.vector.tensor_scalar(out=kt_sb[:, m, :], in0=ps,
                                    scalar1=bk_sb[:, m:m + 1], scalar2=None,
                                    op0=OP.add)
        for tch in range(4):
            for nh2 in range(2):
                ps = pp.tile([128, ST], F32, name='ps')
                for c in range(8):
                    nc.tensor.matmul(
                        ps, lhsT=xt_sb[:, c, tch * 128:(tch + 1) * 128],
                        rhs=w_sb['v'][:, c, nh2 * 512:(nh2 + 1) * 512],
                        start=(c == 0), stop=(c == 7))
                nc.vector.tensor_tensor(
                    out=v_sb[:, tch, nh2 * 512:(nh2 + 1) * 512], in0=ps,
                    in1=bv_bc[:, nh2 * 512:(nh2 + 1) * 512], op=OP.add)

        # ---- attention + output projection, per 128-token block ----
        # Software-pipelined: the next group's scores matmuls are emitted
        # between this group's transposes and its AV matmuls, so the PE has
        # work while the ACT engine copies A^T out of PSUM. The O-projection
        # is split at the half-contraction point: its first 4 c-chunks only
        # need ps_av0 (copied to ot at g==1), so they fill the wait for the
        # second ot copy.
        def emit_scores(b4, g):
            t0 = b4 * 128
            # Heads grouped by parity: every scores matmul in this group
            # reads Q^T/K^T at the SAME partition offset. Mixing partition
            # offsets across matmuls that write one PSUM bank wedges the
            # device (HW/codegen bug), so each bank sees one offset only.
            parity = g % 2
            base = (g // 2) * 8
            heads = [base + parity + 2 * i for i in range(4)]
            off = parity * 64
            ps_sc = psc.tile([128, 4, 128], F32, name='ps_sc')
            for i, hh in enumerate(heads):
                m = hh // 2
                nc.tensor.matmul(
                    ps_sc[:, i, :],
                    lhsT=qt_sb[off:off + 64, m, t0:t0 + 128],
                    rhs=kt_sb[off:off + 64, m, t0:t0 + 128],
                    start=True, stop=True)
            return ps_sc, heads, off

        sc_pending = emit_scores(0, 0)
        for b4 in range(4):
            t0 = b4 * 128
            ps_av0 = pav.tile([128, 4, 128], F32, name='ps_av0')
            ps_av1 = pav.tile([128, 4, 128], F32, name='ps_av1')
            for g in range(4):
                ps_sc, heads, off = sc_pending
                e_sb = apool.tile([128, 4, 128], BF16, name='e')
                nc.scalar.activation(e_sb, ps_sc, AF.Exp)
                stat = apool.tile([128, 8], F32, name='stat')
                nc.vector.reduce_sum(out=stat[:, 0:4], in_=e_sb, axis=AX.X)
                nc.vector.reciprocal(stat[:, 4:8], stat[:, 0:4])
                nc.vector.tensor_tensor(out=e_sb, in0=e_sb,
                                        in1=stat[:, 4:8].to_broadcast((128, 4, 128)),
                                        op=OP.mult)
                ps_at = pat.tile([128, 4, 128], BF16, name='ps_at')
                for i in range(4):
                    nc.tensor.transpose(ps_at[:, i, :], e_sb[:, i, :], ident)
                if g < 3:
                    sc_pending = emit_scores(b4, g + 1)
                elif b4 + 1 < 4:
                    sc_pending = emit_scores(b4 + 1, 0)
                at_sb = apool.tile([128, 4, 128], BF16, name='at')
                nc.scalar.copy(at_sb, ps_at)
                for i, hh in enumerate(heads):
                    g2 = hh // 2
                    ps_av = ps_av0 if g2 < 4 else ps_av1
                    nc.tensor.matmul(
                        ps_av[off:off + 64, g2 % 4, :],
                        lhsT=v_sb[:, b4, hh * 64:(hh + 1) * 64],
                        rhs=at_sb[:, i, :],
                        start=True, stop=True)
                if g == 1:
                    ot_sb = otpool.tile([128, 8, 128], BF16, name='ot')
                    nc.scalar.copy(ot_sb[:, 0:4, :], ps_av0)
                elif g == 3:
                    nc.scalar.copy(ot_sb[:, 4:8, :], ps_av1)

            ps_o0 = pp.tile([128, ST], F32, name='ps')
            ps_o1 = pp.tile([128, ST], F32, name='ps')
            for nh2, ps_o in ((0, ps_o0), (1, ps_o1)):
                for c in range(4):
                    nc.tensor.matmul(
                        ps_o, lhsT=ot_sb[:, c, :],
                        rhs=w_sb['o'][:, c, nh2 * 512:(nh2 + 1) * 512],
                        start=(c == 0), stop=False)
            for nh2, ps_o in ((0, ps_o0), (1, ps_o1)):
                for c in range(4, 8):
                    nc.tensor.matmul(
                        ps_o, lhsT=ot_sb[:, c, :],
                        rhs=w_sb['o'][:, c, nh2 * 512:(nh2 + 1) * 512],
                        start=False, stop=(c == 7))
                out_sb = opool.tile([128, 512], BF16, name='outsb')
                nc.vector.tensor_tensor(
                    out=out_sb, in0=ps_o,
                    in1=bo_bc[:, nh2 * 512:(nh2 + 1) * 512], op=OP.add)
                nc.sync.dma_start(
                    out=out_d.ap()[s * ST + t0: s * ST + t0 + 128,
                                   nh2 * 512:(nh2 + 1) * 512],
                    in_=out_sb)

_NC_CACHE = []


def _get_nc():
    if not _NC_CACHE:
        _NC_CACHE.append(build_bass())
    return _NC_CACHE[0]


def shard_inputs(x, Wq, bq, Wk, bk, Wv, bv, Wo, bo):
    x = np.asarray(x, dtype=np.float32)
    B, S, _ = x.shape
    xf = np.ascontiguousarray(x.reshape(B * S, D))
    assert B * S == N_CORES * TOK

    def qk_layout(W):
        # [in=1024, out=1024] -> [p=128, m=8, c=8, 128]; element [p,m,c,j] =
        # W[c*128+p, m*128+j]: per-(p,m) rows are contiguous 2KB DMA runs.
        return np.ascontiguousarray(
            np.asarray(W, dtype=np.float16).reshape(8, 128, 8, 128)
            .transpose(1, 2, 0, 3))

    def vo_layout(W):
        # [in, out] -> [p=128, c=8, n=1024]; per-partition rows contiguous.
        return np.ascontiguousarray(
            np.asarray(W, dtype=np.float16).reshape(8, 128, D)
            .transpose(1, 0, 2))

    shared = {
        'wq': qk_layout(Wq),
        'wk': qk_layout(Wk),
        'wv': vo_layout(Wv),
        'wo': vo_layout(Wo),
        'bq': np.ascontiguousarray(np.asarray(bq, dtype=np.float32).reshape(8, 128).T),
        'bk': np.ascontiguousarray(np.asarray(bk, dtype=np.float32).reshape(8, 128).T),
        'bv': np.ascontiguousarray(np.asarray(bv, dtype=np.float32).reshape(1, D)),
        'bo': np.ascontiguousarray(np.asarray(bo, dtype=np.float32).reshape(1, D)),
    }
    in_maps = []
    for c in range(N_CORES):
        # [tok=2048, d=1024] -> [p=128, s=4, c=8, t=512]; element [p,s,ch,t]
        # = x[s*512+t, ch*128+p]: per-(p,s) rows contiguous 8KB DMA runs.
        xs = xf[c * TOK:(c + 1) * TOK, :].astype(np.float16)
        xt = np.ascontiguousarray(
            xs.T.reshape(8, 128, NST, ST).transpose(1, 2, 0, 3))
        in_maps.append({'xt': xt, **shared})
    return (B, S), in_maps


def run(inputs, **spmd_kwargs):
    (B, S), in_maps = shard_inputs(**inputs)
    nc = _get_nc()
    res = run_bass_kernel_spmd(nc, in_maps, list(range(N_CORES)), **spmd_kwargs)
    out = np.concatenate([res.results[c]['out'] for c in range(N_CORES)], axis=0)
    return out.astype(np.float32).reshape(B, S, D), res


def kernel(x, Wq, bq, Wk, bk, Wv, bv, Wo, bo):
    out, _ = run(dict(x=x, Wq=Wq, bq=bq, Wk=Wk, bk=bk,
                      Wv=Wv, bv=bv, Wo=Wo, bo=bo))
    return out
